# revision 25
# baseline (speedup 1.0000x reference)
"""GPT forward (L=6, B=2, T=1024, D=768, H=12, V=50257) on 8 TRN2 NeuronCores.

Sharding: tokens sharded 8-way (each core owns two causally-complementary
128-token blocks of one batch), weights replicated and streamed as bf16,
per-layer fused K+V AllGather (bf16) within each 4-core batch group,
classifier vocab-sharded 8-way after a final hidden-state AllGather (bf16).
Activations are feature-major [D, t]; weight matmuls run with bf16 stationary
(FWL) x f32r moving; attention is fully bf16.  AV is computed feature-major
(out[dk, q]) so no PE transposes are needed.  The program is core-uniform:
per-core differences (token positions, causal masks, vocab slice) enter as
input data.
"""
import os
import numpy as np
from contextlib import ExitStack

import concourse.bass as bass
import concourse.tile as tile
import concourse.mybir as mybir
from concourse import bacc, bass_utils

F32 = mybir.dt.float32
F32R = mybir.dt.float32r
BF16 = mybir.dt.bfloat16
AF = mybir.ActivationFunctionType
OP = mybir.AluOpType

L, B, T, D, H, DK, V = 6, 2, 1024, 768, 12, 64, 50257
NB, TB, TPC = 8, 128, 256
NJ = D // 128                        # 6
NJ1 = 4 * D // 128                   # 24
VCHUNK = 512
NVC = 13
VCP = NVC * VCHUNK                   # 6656
VC = 6283                            # 8*6283 = 50264 >= V
EPS = 1e-5
NMT = 16
NPAR = 8 * NJ + NJ1 + D              # packed per-layer params: 48+24+768
NLAYER = int(os.environ.get("KLAYERS", str(L)))

KB_RANK = [j if j < 4 else 7 - j for j in range(NB)]
KB_HALF = [0 if j < 4 else 1 for j in range(NB)]


def _build():
    nc = bacc.Bacc("TRN2", target_bir_lowering=False, debug=False)

    di = {}
    def din(name, shape, dt=F32R):
        di[name] = nc.dram_tensor(name, shape, dt, kind="ExternalInput")
        return di[name]

    din("x0T", [128, NJ * TPC])
    din("cosT", [128, NJ * TPC], BF16)
    din("sinS", [128, NJ * TPC], BF16)
    din("masks", [NB, 128, TPC], BF16)
    din("onecol", [128, 1])
    din("ones96", [128, NB * H, 2], BF16)
    din("embT", [D, VCP], BF16)
    for nm in ("Wq", "Wk", "Wv", "Wo"):
        din(nm, [L, D, D], BF16)
    din("W1", [L, D, 4 * D], BF16)
    din("W2", [L, 4 * D, D], BF16)
    din("params", [L, 128, NPAR], F32)
    din("lnw_p", [128, NJ], F32)
    din("lnb_p", [128, NJ], F32)

    out_logits = nc.dram_tensor("logits", [NMT * 128, VCP], F32, kind="ExternalOutput")

    with tile.TileContext(nc) as tc, ExitStack() as octx:
        const = octx.enter_context(tc.tile_pool(name="const", bufs=1))
        xpool = octx.enter_context(tc.tile_pool(name="x", bufs=1))
        small = octx.enter_context(tc.tile_pool(name="small", bufs=2))
        bias = octx.enter_context(tc.tile_pool(name="bias", bufs=2))
        pp = octx.enter_context(tc.tile_pool(name="pp", bufs=3, space="PSUM"))
        pp1 = octx.enter_context(tc.tile_pool(name="pp1", bufs=1, space="PSUM"))
        dram = octx.enter_context(tc.tile_pool(name="dram", bufs=2, space="DRAM"))

        t_ones = const.tile([128, 1], F32R, tag="ones")
        nc.sync.dma_start(t_ones[:], di["onecol"].ap())
        t_lnw = const.tile([128, NJ], F32, tag="lnw")
        nc.sync.dma_start(t_lnw[:], di["lnw_p"].ap())
        t_lnb = const.tile([128, NJ], F32, tag="lnb")
        nc.sync.dma_start(t_lnb[:], di["lnb_p"].ap())
        t_eps = const.tile([1, 1], F32, tag="eps")
        nc.gpsimd.memset(t_eps[:], EPS)

        t_x = xpool.tile([128, NJ * TPC], F32R, tag="x")
        nc.sync.dma_start(t_x[:], di["x0T"].ap())

        pcnt = [0]

        def psum2(w=512):
            pcnt[0] += 1
            return pp.tile([128, w], F32, tag="pp", name=f"ps{pcnt[0]}")

        def psum1(w=512):
            pcnt[0] += 1
            return pp1.tile([1, w], F32, tag="pp1", name=f"ps{pcnt[0]}")

        def layernorm(wpool, src, dst, gt, bt):
            """feature-major LN: dst = (src - mean)/std * g + b, per token."""
            t_red = psum1()
            p_s, p_q = t_red[:, 0:TPC], t_red[:, TPC:2 * TPC]
            for j in range(NJ):
                nc.tensor.matmul(p_s, t_ones[:], src[:, j * TPC:(j + 1) * TPC],
                                 start=(j == 0), stop=(j == NJ - 1))
            t_sq = wpool.tile([128, NJ * TPC], F32R, tag="scratch6")
            nc.vector.tensor_tensor(t_sq[:], src[:], src[:], OP.mult)
            for j in range(NJ):
                nc.tensor.matmul(p_q, t_ones[:], t_sq[:, j * TPC:(j + 1) * TPC],
                                 start=(j == 0), stop=(j == NJ - 1))
            # ones vector holds 1/D, so p_s = mean and p_q = E[x^2] directly
            t_mean = small.tile([1, TPC], F32, tag="mean")
            nc.vector.tensor_copy(t_mean[:], p_s)
            t_msq = small.tile([1, TPC], F32, tag="msq")
            nc.vector.tensor_tensor(t_msq[:], t_mean[:], t_mean[:], OP.mult)
            t_var = small.tile([1, TPC], F32, tag="var")
            nc.vector.tensor_tensor(t_var[:], p_q, t_msq[:], OP.subtract)
            t_std = small.tile([1, TPC], F32, tag="std")
            nc.scalar.activation(t_std[:], t_var[:], AF.Sqrt, bias=t_eps[:])
            t_rstd = small.tile([1, TPC], F32, tag="rstd")
            nc.vector.reciprocal(t_rstd[:], t_std[:])
            t_mb = small.tile([128, TPC], F32, tag="mb")
            nc.gpsimd.partition_broadcast(t_mb[:], t_mean[:])
            t_rb = small.tile([128, TPC], F32, tag="rb")
            nc.gpsimd.partition_broadcast(t_rb[:], t_rstd[:])
            for j in range(NJ):
                sl = slice(j * TPC, (j + 1) * TPC)
                nc.vector.tensor_tensor(t_sq[:, sl], src[:, sl], t_mb[:],
                                        OP.subtract)
                nc.vector.tensor_tensor(t_sq[:, sl], t_sq[:, sl], t_rb[:], OP.mult)
                nc.vector.tensor_scalar(dst[:, sl], t_sq[:, sl], gt[:, j:j + 1],
                                        bt[:, j:j + 1], OP.mult, OP.add)

        def rope(wpool, t_q, t_cos, t_sin):
            """in-place RoPE on feature-major bf16 [128, NJ*TPC] tile."""
            t_sw = wpool.tile([128, NJ * TPC], BF16, tag="ropesw")
            W = NJ * TPC
            nc.vector.tensor_copy(t_sw[0:32, 0:W], t_q[32:64, 0:W])
            nc.vector.tensor_copy(t_sw[32:64, 0:W], t_q[0:32, 0:W])
            nc.vector.tensor_copy(t_sw[64:96, 0:W], t_q[96:128, 0:W])
            nc.vector.tensor_copy(t_sw[96:128, 0:W], t_q[64:96, 0:W])
            nc.vector.tensor_tensor(t_sw[:], t_sw[:], t_sin[:], OP.mult)
            nc.vector.tensor_tensor(t_q[:], t_q[:], t_cos[:], OP.mult)
            nc.vector.tensor_tensor(t_q[:], t_q[:], t_sw[:], OP.add)

        def wpass(wsl_pool, wdram, l, nk, rhs, rhs_k_slice, out_fn):
            """out[n] = sum_k W[l,k].T @ rhs_k ; W streamed bf16, psum-resident.
            out_fn(n, ps_ap) evicts a [128, TPC] psum AP for feature-tile n."""
            pss = [psum2() for _ in range(3)]
            pap = lambda n: pss[n // 2][:, (n % 2) * TPC:(n % 2 + 1) * TPC]
            for k in range(nk):
                wk = wsl_pool.tile([128, NJ * 128], BF16, tag="wsl")
                nc.sync.dma_start(wk[:], wdram.ap()[l, k * 128:(k + 1) * 128, :])
                for n in range(NJ):
                    nc.tensor.matmul(pap(n), wk[:, n * 128:(n + 1) * 128],
                                     rhs[:, rhs_k_slice(k)],
                                     start=(k == 0 and n % 2 == 0),
                                     stop=(k == nk - 1),
                                     skip_group_check=True)
            for n in range(NJ):
                out_fn(n, pap(n))

        # ================= phase A: transformer layers =================
        with ExitStack() as actx:
            aconst = actx.enter_context(tc.tile_pool(name="aconst", bufs=1))
            kvp = actx.enter_context(tc.tile_pool(name="kvp", bufs=1))
            wk_ = actx.enter_context(tc.tile_pool(name="work", bufs=1))
            ap_ = actx.enter_context(tc.tile_pool(name="Ap", bufs=2))
            wsl = actx.enter_context(tc.tile_pool(name="wsl", bufs=8))
            h1p = actx.enter_context(tc.tile_pool(name="h1p", bufs=1))
            ppqk = actx.enter_context(tc.tile_pool(name="ppqk", bufs=1, space="PSUM"))
            ppav = actx.enter_context(tc.tile_pool(name="ppav", bufs=2, space="PSUM"))

            t_cos = aconst.tile([128, NJ * TPC], BF16, tag="cos")
            nc.sync.dma_start(t_cos[:], di["cosT"].ap())
            t_sin = aconst.tile([128, NJ * TPC], BF16, tag="sin")
            nc.sync.dma_start(t_sin[:], di["sinS"].ap())
            t_mask = aconst.tile([128, NB * TPC], BF16, tag="mask")
            for kb in range(NB):
                nc.sync.dma_start(t_mask[:, kb * TPC:(kb + 1) * TPC],
                                  di["masks"].ap()[kb])

            t_K = kvp.tile([128, NJ * NB * TB], BF16, tag="K")    # (j, kblk, t)
            t_V = kvp.tile([128, NB * H * 66], BF16, tag="V")     # (kblk, h, dk|one)
            nc.sync.dma_start(
                t_V[:].rearrange("p (b h e) -> p (b h) e", b=NB, h=H)[:, :, 64:66],
                di["ones96"].ap())

            for l in range(NLAYER):
                # --- packed per-layer params, one DMA
                t_par = bias.tile([128, NPAR], F32, tag="par")
                nc.sync.dma_start(t_par[:], di["params"].ap()[l])
                bq_p = t_par[:, 0 * NJ:1 * NJ]
                bk_p = t_par[:, 1 * NJ:2 * NJ]
                bo_p = t_par[:, 2 * NJ:3 * NJ]
                b2_p = t_par[:, 3 * NJ:4 * NJ]
                g_p = t_par[:, 4 * NJ:5 * NJ]
                be_p = t_par[:, 5 * NJ:6 * NJ]
                l2w_p = t_par[:, 6 * NJ:7 * NJ]
                l2b_p = t_par[:, 7 * NJ:8 * NJ]
                b1_p = t_par[:, 8 * NJ:8 * NJ + NJ1]
                bv_bc = t_par[:, 8 * NJ + NJ1:]

                # --- LN1
                t_xn = wk_.tile([128, NJ * TPC], BF16, tag="xn")
                layernorm(wk_, t_x, t_xn, g_p, be_p)

                # --- V projection (token-major) + bias -> bf16
                t_vc = wk_.tile([128, 2 * D], BF16, tag="vc")
                psv = [psum2() for _ in range(3)]
                vap = lambda i: psv[i // 2][:, (i % 2) * TPC:(i % 2 + 1) * TPC]
                for k in range(NJ):
                    wvk = wsl.tile([128, NJ * 128], BF16, tag="wsl")
                    nc.sync.dma_start(wvk[:], di["Wv"].ap()[l, k * 128:(k + 1) * 128, :])
                    for tt in range(2):
                        lhs = t_xn[:, k * TPC + tt * TB: k * TPC + (tt + 1) * TB]
                        for s3 in range(3):
                            i6 = tt * 3 + s3
                            nc.tensor.matmul(
                                vap(i6), lhs, wvk[:, s3 * 256:(s3 + 1) * 256],
                                start=(k == 0 and i6 % 2 == 0),
                                stop=(k == NJ - 1), skip_group_check=True)
                for tt in range(2):
                    for s3 in range(3):
                        nc.vector.tensor_tensor(
                            t_vc[:, tt * D + s3 * 256: tt * D + (s3 + 1) * 256],
                            vap(tt * 3 + s3), bv_bc[:, s3 * 256:(s3 + 1) * 256], OP.add)

                # --- K projection (feature-major) + bias -> bf16, RoPE
                t_k = wk_.tile([128, NJ * TPC], BF16, tag="k")
                wpass(wsl, di["Wk"], l, NJ, t_xn,
                      lambda k: slice(k * TPC, (k + 1) * TPC),
                      lambda n, p: nc.scalar.activation(
                          t_k[:, n * TPC:(n + 1) * TPC], p, AF.Identity,
                          bias=bk_p[:, n:n + 1]))
                rope(wk_, t_k, t_cos, t_sin)

                # --- fused K+V AllGather (bf16), SBUF-verbatim layouts
                kvag_in = dram.tile([128, 2 * NJ * TPC], BF16, tag="kvag_in")
                nc.gpsimd.dma_start(kvag_in[:, 0:NJ * TPC], t_k[:])
                nc.gpsimd.dma_start(kvag_in[:, NJ * TPC:2 * NJ * TPC], t_vc[:])
                kvag_out = dram.tile([4 * 128, 2 * NJ * TPC], BF16, tag="kvag_out")
                nc.gpsimd.collective_compute(
                    "AllGather", OP.bypass,
                    replica_groups=[[0, 1, 2, 3], [4, 5, 6, 7]],
                    ins=[kvag_in[:].opt()], outs=[kvag_out[:].opt()])

                # --- Q projection + RoPE (overlaps the collective)
                t_q = wk_.tile([128, NJ * TPC], BF16, tag="q")
                wpass(wsl, di["Wq"], l, NJ, t_xn,
                      lambda k: slice(k * TPC, (k + 1) * TPC),
                      lambda n, p: nc.scalar.activation(
                          t_q[:, n * TPC:(n + 1) * TPC], p, AF.Identity,
                          bias=bq_p[:, n:n + 1]))
                rope(wk_, t_q, t_cos, t_sin)

                # --- load gathered K (feature-major) and V (token-major)
                kv4 = t_K[:].rearrange("p (j b t) -> p j b t", j=NJ, b=NB)
                vv4 = t_V[:].rearrange("p (b h e) -> p b h e", b=NB, h=H)
                for j in range(NB):
                    r, hf = KB_RANK[j], KB_HALF[j]
                    rrows = kvag_out[r * 128:(r + 1) * 128, :]
                    srck = rrows[:, 0:NJ * TPC].rearrange(
                        "p (j2 t) -> p j2 t", j2=NJ)[:, :, hf * TB:(hf + 1) * TB]
                    nc.sync.dma_start(kv4[:, :, j, :], srck)
                    srcv = rrows[:, NJ * TPC + hf * D:
                                 NJ * TPC + (hf + 1) * D].rearrange(
                        "p (h e) -> p h e", h=H)
                    eng = nc.scalar if j % 2 == 0 else nc.gpsimd
                    eng.dma_start(vv4[:, j, :, 0:64], srcv)

                # --- attention (feature-major AV; no transposes)
                t_attT = wk_.tile([128, NJ * TPC], BF16, tag="attT")
                for h in range(H):
                    jq, po = h // 2, 64 * (h % 2)
                    t_A = ap_.tile([128, NB * TPC], BF16, tag="A")
                    for half in range(2):
                        ps_qk = ppqk.tile([128, 4 * TPC], F32, tag="qk",
                                          name=f"qk{l}_{h}_{half}")
                        for k4 in range(4):
                            kb = half * 4 + k4
                            nc.tensor.matmul(
                                ps_qk[:, k4 * TPC:(k4 + 1) * TPC],
                                t_K[po:po + 64,
                                    (jq * NB + kb) * TB:(jq * NB + kb + 1) * TB],
                                t_q[po:po + 64, jq * TPC:(jq + 1) * TPC])
                        nc.scalar.activation(
                            t_A[:, half * 4 * TPC:(half + 1) * 4 * TPC],
                            ps_qk[:], AF.Exp, scale=0.125)
                    eng = nc.vector if h % 2 == 0 else nc.gpsimd
                    eng.tensor_tensor(t_A[:], t_A[:], t_mask[:], OP.mult)
                    ps_av = ppav.tile([128, TPC], F32, tag="av", name=f"av{l}_{h}")
                    for kb in range(NB):
                        nc.tensor.matmul(
                            ps_av[0:66, :],
                            t_V[:, (kb * H + h) * 66:(kb * H + h) * 66 + 66],
                            t_A[:, kb * TPC:(kb + 1) * TPC],
                            start=(kb == 0), stop=(kb == NB - 1))
                    t_rl = small.tile([1, TPC], F32, tag="rl")
                    nc.vector.reciprocal(t_rl[:], ps_av[64:65, :])
                    t_rb2 = small.tile([64, TPC], F32, tag="rb2")
                    nc.gpsimd.partition_broadcast(t_rb2[:], t_rl[:])
                    nc.vector.tensor_tensor(
                        t_attT[po:po + 64, jq * TPC:(jq + 1) * TPC],
                        ps_av[0:64, :], t_rb2[:], OP.mult)

                # --- Wo + residual
                t_mo = wk_.tile([128, NJ * TPC], F32, tag="mmout")
                wpass(wsl, di["Wo"], l, NJ, t_attT,
                      lambda k: slice(k * TPC, (k + 1) * TPC),
                      lambda n, p: nc.scalar.activation(
                          t_mo[:, n * TPC:(n + 1) * TPC], p, AF.Identity,
                          bias=bo_p[:, n:n + 1]))
                nc.vector.tensor_tensor(t_x[:], t_x[:], t_mo[:], OP.add)

                # --- LN2 + MLP
                t_xn2 = wk_.tile([128, NJ * TPC], BF16, tag="xn2")
                layernorm(wk_, t_x, t_xn2, l2w_p, l2b_p)

                t_h1 = h1p.tile([128, NJ1 * TPC], BF16, tag="h1")
                for g in range(4):
                    psg = [psum2() for _ in range(3)]
                    gap = lambda n: psg[n // 2][:, (n % 2) * TPC:(n % 2 + 1) * TPC]
                    for k in range(NJ):
                        w1k = wsl.tile([128, NJ * 128], BF16, tag="wsl")
                        nc.sync.dma_start(
                            w1k[:], di["W1"].ap()[l, k * 128:(k + 1) * 128,
                                                  g * D:(g + 1) * D])
                        for n in range(NJ):
                            nc.tensor.matmul(
                                gap(n), w1k[:, n * 128:(n + 1) * 128],
                                t_xn2[:, k * TPC:(k + 1) * TPC],
                                start=(k == 0 and n % 2 == 0),
                                stop=(k == NJ - 1), skip_group_check=True)
                    for n in range(NJ):
                        gn = g * NJ + n
                        nc.scalar.activation(
                            t_h1[:, gn * TPC:(gn + 1) * TPC], gap(n), AF.Gelu,
                            bias=b1_p[:, gn:gn + 1])

                wpass(wsl, di["W2"], l, NJ1, t_h1,
                      lambda k: slice(k * TPC, (k + 1) * TPC),
                      lambda n, p: nc.scalar.activation(
                          t_mo[:, n * TPC:(n + 1) * TPC], p, AF.Identity,
                          bias=b2_p[:, n:n + 1]))
                nc.vector.tensor_tensor(t_x[:], t_x[:], t_mo[:], OP.add)

        # ================= phase B: final LN + classifier =================
        with ExitStack() as bctx:
            bw = bctx.enter_context(tc.tile_pool(name="bw", bufs=1))
            hallp = bctx.enter_context(tc.tile_pool(name="hall", bufs=1))
            embp = bctx.enter_context(tc.tile_pool(name="embp", bufs=8))
            ppc = bctx.enter_context(tc.tile_pool(name="ppc", bufs=2, space="PSUM"))

            t_hT = bw.tile([128, NJ * TPC], BF16, tag="hT")
            layernorm(bw, t_x, t_hT, t_lnw, t_lnb)
            hag_in = dram.tile([128, NJ * TPC], BF16, tag="hag_in")
            nc.gpsimd.dma_start(hag_in[:], t_hT[:])
            hag_out = dram.tile([8 * 128, NJ * TPC], BF16, tag="hag_out",
                                addr_space="Shared")
            nc.gpsimd.collective_compute(
                "AllGather", OP.bypass,
                replica_groups=[[0, 1, 2, 3, 4, 5, 6, 7]],
                ins=[hag_in[:].opt()], outs=[hag_out[:].opt()])

            t_hall = hallp.tile([128, 8 * NJ * TPC], BF16, tag="hall")
            for r in range(8):
                eng = (nc.sync, nc.scalar, nc.gpsimd)[r % 3]
                eng.dma_start(t_hall[:, r * NJ * TPC:(r + 1) * NJ * TPC],
                              hag_out[r * 128:(r + 1) * 128, :])

            for vc in range(NVC):
                ets = []
                for k in range(NJ):
                    et = embp.tile([128, VCHUNK], BF16, tag="emb", name=f"emb{vc}_{k}")
                    nc.sync.dma_start(
                        et[:], di["embT"].ap()[k * 128:(k + 1) * 128,
                                               vc * VCHUNK:(vc + 1) * VCHUNK])
                    ets.append(et)
                for mt in range(NMT):
                    beta, j = divmod(mt, NB)
                    r, hf = beta * 4 + KB_RANK[j], KB_HALF[j]
                    pcnt[0] += 1
                    pc = ppc.tile([128, VCHUNK], F32, tag="ppc",
                                  name=f"pc{pcnt[0]}")
                    for k in range(NJ):
                        nc.tensor.matmul(
                            pc[:],
                            t_hall[:, (r * NJ + k) * TPC + hf * TB:
                                   (r * NJ + k) * TPC + (hf + 1) * TB],
                            ets[k][:], start=(k == 0), stop=(k == NJ - 1))
                    so = embp.tile([128, VCHUNK], F32, tag="clso",
                                   name=f"clso{vc}_{mt}")
                    if mt % 2 == 0:
                        nc.scalar.activation(so[:], pc[:], AF.Copy)
                    else:
                        nc.vector.tensor_copy(so[:], pc[:])
                    nc.sync.dma_start(
                        out_logits.ap()[mt * 128:(mt + 1) * 128,
                                        vc * VCHUNK:(vc + 1) * VCHUNK], so[:])

    nc.compile()
    return nc


_NC = None


def _get_nc():
    global _NC
    if _NC is None:
        _NC = _build()
    return _NC


def _pack_fm(M):
    """[768, t] feature-major -> [128, 6*t] tile layout (row d=128*j+p)."""
    t = M.shape[1]
    return np.ascontiguousarray(
        M.reshape(NJ, 128, t).transpose(1, 0, 2).reshape(128, NJ * t),
        dtype=np.float32)


def _pack_pp(v):
    """per-feature vector [D'] -> per-partition [128, D'/128]."""
    return np.ascontiguousarray(v.reshape(-1, 128).T, dtype=np.float32)


def _prep_in_maps(inputs):
    import ml_dtypes
    bf16 = ml_dtypes.bfloat16
    f32 = lambda a: np.ascontiguousarray(a, dtype=np.float32)
    emb = f32(inputs["emb"])
    tok = np.asarray(inputs["input_token"]).astype(np.int64)
    x0 = emb[tok]                                    # [B, T, D]

    # packed per-layer params [L, 128, NPAR]
    params = np.zeros((L, 128, NPAR), np.float32)
    for li in range(L):
        cols = []
        for src in ("bq", "bk", "bo", "b2", "gamma", "beta", "ln2_w", "ln2_b"):
            cols.append(_pack_pp(f32(inputs[src][li])))
        cols.append(_pack_pp(f32(inputs["b1"][li])))
        cols.append(np.tile(f32(inputs["bv"][li])[None, :], (128, 1)))
        params[li] = np.concatenate(cols, axis=1)

    shared = {
        "Wq": np.ascontiguousarray(f32(inputs["Wq"]).astype(bf16)),
        "Wk": np.ascontiguousarray(f32(inputs["Wk"]).astype(bf16)),
        "Wv": np.ascontiguousarray(f32(inputs["Wv"]).astype(bf16)),
        "Wo": np.ascontiguousarray(f32(inputs["Wo"]).astype(bf16)),
        "W1": np.ascontiguousarray(f32(inputs["W1"]).astype(bf16)),
        "W2": np.ascontiguousarray(f32(inputs["W2"]).astype(bf16)),
        "params": params,
        "onecol": np.full((128, 1), 1.0 / D, np.float32),
        "ones96": np.ones((128, NB * H, 2), bf16),
        "lnw_p": _pack_pp(f32(inputs["ln_w"])),
        "lnb_p": _pack_pp(f32(inputs["ln_b"])),
    }

    inv = 1.0 / (10000.0 ** (np.arange(0, DK, 2, dtype=np.float32) / DK))
    embT_full = emb.T                                # [D, V]
    vpad = np.zeros((D, 8 * VC), np.float32)
    vpad[:, :V] = embT_full

    # diag causal mask (key-major): M[kt, qt] = 1 if kt <= qt
    diag = np.tril(np.ones((TB, TB), np.float32)).T

    in_maps = []
    for c in range(8):
        beta, i = divmod(c, 4)
        qb = (i, 7 - i)
        pos = np.concatenate([np.arange(qb[0] * TB, (qb[0] + 1) * TB),
                              np.arange(qb[1] * TB, (qb[1] + 1) * TB)])
        xc = x0[beta, pos]                           # [256, D]
        m = dict(shared)
        m["x0T"] = _pack_fm(xc.T)

        fr = pos[:, None].astype(np.float32) * inv[None, :]      # [256, 32]
        ang = np.concatenate([fr, fr], 1)                        # [256, 64]
        cosT = np.cos(ang).T                                     # [64, 256]
        sinT = np.sin(ang).T
        sinSg = sinT.copy()
        sinSg[:32] = -sinT[:32]
        m["cosT"] = np.ascontiguousarray(np.tile(cosT, (2, NJ))).astype(bf16)
        m["sinS"] = np.ascontiguousarray(np.tile(sinSg, (2, NJ))).astype(bf16)

        masks = np.zeros((NB, 128, TPC), np.float32)
        for kb in range(NB):
            for qi in range(2):
                blk = qb[qi]
                if kb < blk:
                    masks[kb, :, qi * TB:(qi + 1) * TB] = 1.0
                elif kb == blk:
                    masks[kb, :, qi * TB:(qi + 1) * TB] = diag
        m["masks"] = masks.astype(bf16)

        esl = np.zeros((D, VCP), np.float32)
        esl[:, :VC] = vpad[:, c * VC:(c + 1) * VC]
        m["embT"] = esl.astype(bf16)
        in_maps.append(m)

    return in_maps


def _assemble(res):
    out = np.empty((B, T, 8 * VC), np.float32)
    for c in range(8):
        lr = res.results[c]["logits"].reshape(B, T, VCP)
        out[:, :, c * VC:(c + 1) * VC] = lr[:, :, :VC]
    return np.ascontiguousarray(out[:, :, :V])


def kernel(**inputs):
    nc = _get_nc()
    in_maps = _prep_in_maps(inputs)
    res = bass_utils.run_bass_kernel_spmd(nc, in_maps, core_ids=list(range(8)))
    return _assemble(res)


def run_traced(inputs, tmpdir):
    nc = _get_nc()
    in_maps = _prep_in_maps(inputs)
    return bass_utils.run_bass_kernel_spmd(
        nc, in_maps, core_ids=list(range(8)), trace=True, tmpdir=tmpdir)


# revision 26
# speedup vs baseline: 1.1695x; 1.1695x over previous
"""GPT forward (L=6, B=2, T=1024, D=768, H=12, V=50257) on 8 TRN2 NeuronCores.

Sharding: tokens sharded 8-way (each core owns two causally-complementary
128-token blocks of one batch), weights replicated and streamed bf16 with
pair-packed k-tiles (3KB DMA runs), per-layer split K then V AllGathers
(bf16) within each 4-core batch group; V's AllGather overlaps the QK/exp
phase, which only needs K.  Classifier is vocab-sharded 8-way after a final
bf16 hidden-state AllGather.  Activations are feature-major [D, t]; all
matmuls are bf16 x bf16 into fp32 PSUM.  AV is feature-major (out[dk, q]) so
no PE transposes are needed; softmax denominators ride as an extra ones
column in the V payload.  The program is core-uniform: per-core differences
(token positions, causal masks, vocab slice) enter as input data.
"""
import os
import numpy as np
from contextlib import ExitStack

import concourse.bass as bass
import concourse.tile as tile
import concourse.mybir as mybir
from concourse import bacc, bass_utils

F32 = mybir.dt.float32
F32R = mybir.dt.float32r
BF16 = mybir.dt.bfloat16
AF = mybir.ActivationFunctionType
OP = mybir.AluOpType

L, B, T, D, H, DK, V = 6, 2, 1024, 768, 12, 64, 50257
NB, TB, TPC = 8, 128, 256
NJ = D // 128                        # 6
NJ1 = 4 * D // 128                   # 24
VCHUNK = 512
NVC = 13
VCP = NVC * VCHUNK                   # 6656
VC = 6283                            # 8*6283 = 50264 >= V
EPS = 1e-5
NMT = 16
VW = H * 66                          # 792: one half's V payload incl ones
NPAR = 8 * NJ + NJ1 + D              # packed per-layer params: 48+24+768
NLAYER = int(os.environ.get("KLAYERS", str(L)))

KB_RANK = [j if j < 4 else 7 - j for j in range(NB)]
KB_HALF = [0 if j < 4 else 1 for j in range(NB)]


def _build():
    nc = bacc.Bacc("TRN2", target_bir_lowering=False, debug=False)

    di = {}
    def din(name, shape, dt=F32R):
        di[name] = nc.dram_tensor(name, shape, dt, kind="ExternalInput")
        return di[name]

    din("x0T", [128, NJ * TPC])
    din("cosQ", [128, NJ * TPC], BF16)
    din("sinQ", [128, NJ * TPC], BF16)
    din("cosK", [128, NJ * TPC], BF16)
    din("sinK", [128, NJ * TPC], BF16)
    din("masks", [NB, 128, TPC], BF16)
    din("onecol", [128, 1])
    din("ones_v", [128, 2 * H, 2], BF16)
    din("embT", [D, VCP], BF16)
    for nm in ("Wq", "Wk", "Wv", "Wo"):
        din(nm, [L, D // 2, 2 * D], BF16)      # pair-packed k-tiles
    din("W1", [L, D // 2, 4, 2 * D], BF16)     # (kp, p, group, half*n)
    din("W2", [L, 2 * D, 2 * D], BF16)
    din("params", [L, 128, NPAR], F32)
    din("lnw_p", [128, NJ], F32)
    din("lnb_p", [128, NJ], F32)

    out_logits = nc.dram_tensor("logits", [NMT * 128, VCP], F32, kind="ExternalOutput")

    with tile.TileContext(nc) as tc, ExitStack() as octx:
        const = octx.enter_context(tc.tile_pool(name="const", bufs=1))
        xpool = octx.enter_context(tc.tile_pool(name="x", bufs=1))
        small = octx.enter_context(tc.tile_pool(name="small", bufs=2))
        bias = octx.enter_context(tc.tile_pool(name="bias", bufs=2))
        pp = octx.enter_context(tc.tile_pool(name="pp", bufs=3, space="PSUM"))
        dram = octx.enter_context(tc.tile_pool(name="dram", bufs=2, space="DRAM"))

        t_ones = const.tile([128, 1], F32R, tag="ones")
        nc.sync.dma_start(t_ones[:], di["onecol"].ap())
        t_lnw = const.tile([128, NJ], F32, tag="lnw")
        nc.sync.dma_start(t_lnw[:], di["lnw_p"].ap())
        t_lnb = const.tile([128, NJ], F32, tag="lnb")
        nc.sync.dma_start(t_lnb[:], di["lnb_p"].ap())
        t_eps = const.tile([1, 1], F32, tag="eps")
        nc.gpsimd.memset(t_eps[:], EPS)

        t_x = xpool.tile([128, NJ * TPC], F32R, tag="x")
        nc.sync.dma_start(t_x[:], di["x0T"].ap())

        pcnt = [0]

        def psum2():
            pcnt[0] += 1
            return pp.tile([128, 512], F32, tag="pp", name=f"ps{pcnt[0]}")

        def layernorm(wpool, src, dst, gt, bt):
            """feature-major LN: dst = (src - mean)/std * g + b, per token."""
            pcnt[0] += 1
            t_red = pp.tile([1, 512], F32, tag="pp", name=f"red{pcnt[0]}")
            p_s, p_q = t_red[:, 0:TPC], t_red[:, TPC:2 * TPC]
            for j in range(NJ):
                nc.tensor.matmul(p_s, t_ones[:], src[:, j * TPC:(j + 1) * TPC],
                                 start=(j == 0), stop=(j == NJ - 1))
            t_sq = wpool.tile([128, NJ * TPC], F32R, tag="scratch6")
            nc.vector.tensor_tensor(t_sq[:], src[:], src[:], OP.mult)
            for j in range(NJ):
                nc.tensor.matmul(p_q, t_ones[:], t_sq[:, j * TPC:(j + 1) * TPC],
                                 start=(j == 0), stop=(j == NJ - 1),
                                 skip_group_check=True)
            # ones vector holds 1/D, so p_s = mean and p_q = E[x^2] directly
            t_mean = small.tile([1, TPC], F32, tag="mean")
            nc.vector.tensor_copy(t_mean[:], p_s)
            t_msq = small.tile([1, TPC], F32, tag="msq")
            nc.vector.tensor_tensor(t_msq[:], t_mean[:], t_mean[:], OP.mult)
            t_var = small.tile([1, TPC], F32, tag="var")
            nc.vector.tensor_tensor(t_var[:], p_q, t_msq[:], OP.subtract)
            t_std = small.tile([1, TPC], F32, tag="std")
            nc.scalar.activation(t_std[:], t_var[:], AF.Sqrt, bias=t_eps[:])
            t_rstd = small.tile([1, TPC], F32, tag="rstd")
            nc.vector.reciprocal_approx_fast(t_rstd[:], t_std[:])
            t_mb = small.tile([128, TPC], F32, tag="mb")
            nc.gpsimd.partition_broadcast(t_mb[:], t_mean[:])
            t_rb = small.tile([128, TPC], F32, tag="rb")
            nc.gpsimd.partition_broadcast(t_rb[:], t_rstd[:])
            for j in range(NJ):
                sl = slice(j * TPC, (j + 1) * TPC)
                eng = nc.vector if j % 2 == 0 else nc.gpsimd
                eng.tensor_tensor(t_sq[:, sl], src[:, sl], t_mb[:], OP.subtract)
                eng.tensor_tensor(t_sq[:, sl], t_sq[:, sl], t_rb[:], OP.mult)
                eng.tensor_scalar(dst[:, sl], t_sq[:, sl], gt[:, j:j + 1],
                                  bt[:, j:j + 1], OP.mult, OP.add)

        def rope(wpool, t_r, t_cos, t_sin):
            """in-place RoPE on feature-major bf16 [128, NJ*TPC] tile."""
            t_sw = wpool.tile([128, NJ * TPC], BF16, tag="ropesw")
            W = NJ * TPC
            nc.vector.tensor_copy(t_sw[0:32, 0:W], t_r[32:64, 0:W])
            nc.vector.tensor_copy(t_sw[32:64, 0:W], t_r[0:32, 0:W])
            nc.vector.tensor_copy(t_sw[64:96, 0:W], t_r[96:128, 0:W])
            nc.vector.tensor_copy(t_sw[96:128, 0:W], t_r[64:96, 0:W])
            nc.vector.tensor_tensor(t_sw[:], t_sw[:], t_sin[:], OP.mult)
            nc.vector.tensor_tensor(t_r[:], t_r[:], t_cos[:], OP.mult)
            nc.vector.tensor_tensor(t_r[:], t_r[:], t_sw[:], OP.add)

        def wpass(wsl_pool, wdram, l, nkp, rhs, rhs_k_slice, out_fn):
            """out[n] = sum_k W[l,k].T @ rhs_k with pair-packed weight tiles.
            wdram tile kp holds k-blocks (2kp, 2kp+1) side by side [128, 2N].
            out_fn(n, ps_ap) evicts a [128, TPC] psum AP for feature-tile n."""
            pss = [psum2() for _ in range(3)]
            pap = lambda n: pss[n // 2][:, (n % 2) * TPC:(n % 2 + 1) * TPC]
            for kp in range(nkp):
                wk = wsl_pool.tile([128, 2 * NJ * 128], BF16, tag="wsl")
                nc.sync.dma_start(wk[:], wdram.ap()[l, kp * 128:(kp + 1) * 128, :])
                for half in range(2):
                    k = 2 * kp + half
                    for n in range(NJ):
                        nc.tensor.matmul(
                            pap(n),
                            wk[:, half * D + n * 128: half * D + (n + 1) * 128],
                            rhs[:, rhs_k_slice(k)],
                            start=(k == 0 and n % 2 == 0),
                            stop=(k == 2 * nkp - 1),
                            skip_group_check=True)
            for n in range(NJ):
                out_fn(n, pap(n))

        # ================= phase A: transformer layers =================
        with ExitStack() as actx:
            aconst = actx.enter_context(tc.tile_pool(name="aconst", bufs=1))
            kvp = actx.enter_context(tc.tile_pool(name="kvp", bufs=1))
            wk_ = actx.enter_context(tc.tile_pool(name="work", bufs=1))
            ap_ = actx.enter_context(tc.tile_pool(name="Ap", bufs=12))
            wsl = actx.enter_context(tc.tile_pool(name="wsl", bufs=6))
            h1p = actx.enter_context(tc.tile_pool(name="h1p", bufs=1))
            ppqk = actx.enter_context(
                tc.tile_pool(name="ppqk", bufs=2, space="PSUM"))
            ppav = actx.enter_context(
                tc.tile_pool(name="ppav", bufs=1, space="PSUM"))

            t_cosQ = aconst.tile([128, NJ * TPC], BF16, tag="cosQ")
            nc.sync.dma_start(t_cosQ[:], di["cosQ"].ap())
            t_sinQ = aconst.tile([128, NJ * TPC], BF16, tag="sinQ")
            nc.sync.dma_start(t_sinQ[:], di["sinQ"].ap())
            t_cosK = aconst.tile([128, NJ * TPC], BF16, tag="cosK")
            nc.sync.dma_start(t_cosK[:], di["cosK"].ap())
            t_sinK = aconst.tile([128, NJ * TPC], BF16, tag="sinK")
            nc.sync.dma_start(t_sinK[:], di["sinK"].ap())
            t_mask = aconst.tile([128, NB * TPC], BF16, tag="mask")
            for kb in range(NB):
                nc.sync.dma_start(t_mask[:, kb * TPC:(kb + 1) * TPC],
                                  di["masks"].ap()[kb])

            t_K = kvp.tile([128, NB * NJ * TB], BF16, tag="K")    # (slot, j, t)
            t_V = kvp.tile([128, NB * VW], BF16, tag="V")         # (slot, h, dk|1|1)
            # producer-side V payload, hf-major with inline ones columns
            t_vc2 = kvp.tile([128, 2 * VW], BF16, tag="vc2")
            nc.sync.dma_start(
                t_vc2[:].rearrange("p (g e) -> p g e", g=2 * H)[:, :, 64:66],
                di["ones_v"].ap())

            for l in range(NLAYER):
                # --- packed per-layer params, one DMA
                t_par = bias.tile([128, NPAR], F32, tag="par")
                nc.sync.dma_start(t_par[:], di["params"].ap()[l])
                bq_p = t_par[:, 0 * NJ:1 * NJ]
                bk_p = t_par[:, 1 * NJ:2 * NJ]
                bo_p = t_par[:, 2 * NJ:3 * NJ]
                b2_p = t_par[:, 3 * NJ:4 * NJ]
                g_p = t_par[:, 4 * NJ:5 * NJ]
                be_p = t_par[:, 5 * NJ:6 * NJ]
                l2w_p = t_par[:, 6 * NJ:7 * NJ]
                l2b_p = t_par[:, 7 * NJ:8 * NJ]
                b1_p = t_par[:, 8 * NJ:8 * NJ + NJ1]
                bv_bc = t_par[:, 8 * NJ + NJ1:]

                # --- LN1
                t_xn = wk_.tile([128, NJ * TPC], BF16, tag="xn")
                layernorm(wk_, t_x, t_xn, g_p, be_p)

                # --- K projection (evict hf-major) + RoPE, then K AllGather
                t_k = wk_.tile([128, NJ * TPC], BF16, tag="k")
                kview = t_k[:].rearrange("p (hf j t) -> p hf j t", hf=2, j=NJ)
                wpass(wsl, di["Wk"], l, NJ // 2, t_xn,
                      lambda k: slice(k * TPC, (k + 1) * TPC),
                      lambda n, p: nc.scalar.activation(
                          kview[:, :, n, :], p, AF.Identity,
                          bias=bk_p[:, n:n + 1]))
                rope(wk_, t_k, t_cosK, t_sinK)
                kag_in = dram.tile([128, NJ * TPC], BF16, tag="kag_in")
                nc.gpsimd.dma_start(kag_in[:], t_k[:])
                kag_out = dram.tile([4 * 128, NJ * TPC], BF16, tag="kag_out")
                nc.gpsimd.collective_compute(
                    "AllGather", OP.bypass,
                    replica_groups=[[0, 1, 2, 3], [4, 5, 6, 7]],
                    ins=[kag_in[:].opt()], outs=[kag_out[:].opt()])

                # --- V projection (token-major) + bias -> t_vc2, V AllGather
                psv = [psum2() for _ in range(3)]
                vap = lambda i: psv[i // 2][:, (i % 2) * TPC:(i % 2 + 1) * TPC]
                for kp in range(NJ // 2):
                    wvk = wsl.tile([128, 2 * NJ * 128], BF16, tag="wsl")
                    nc.sync.dma_start(
                        wvk[:], di["Wv"].ap()[l, kp * 128:(kp + 1) * 128, :])
                    for half in range(2):
                        k = 2 * kp + half
                        for tt in range(2):
                            lhs = t_xn[:, k * TPC + tt * TB: k * TPC + (tt + 1) * TB]
                            for s3 in range(3):
                                i6 = tt * 3 + s3
                                nc.tensor.matmul(
                                    vap(i6), lhs,
                                    wvk[:, half * D + s3 * 256:
                                        half * D + (s3 + 1) * 256],
                                    start=(k == 0 and i6 % 2 == 0),
                                    stop=(k == NJ - 1), skip_group_check=True)
                vc2v = t_vc2[:].rearrange("p (hf g e) -> p hf g e", hf=2, g=H)
                for tt in range(2):
                    for s3 in range(3):
                        nc.vector.tensor_tensor(
                            vc2v[:, tt, s3 * 4:(s3 + 1) * 4, 0:64],
                            vap(tt * 3 + s3).rearrange("p (g e) -> p g e", g=4),
                            bv_bc[:, s3 * 256:(s3 + 1) * 256]
                            .rearrange("p (g e) -> p g e", g=4), OP.add)
                vag_in = dram.tile([128, 2 * VW], BF16, tag="vag_in")
                nc.gpsimd.dma_start(vag_in[:], t_vc2[:])
                vag_out = dram.tile([4 * 128, 2 * VW], BF16, tag="vag_out")
                nc.gpsimd.collective_compute(
                    "AllGather", OP.bypass,
                    replica_groups=[[0, 1, 2, 3], [4, 5, 6, 7]],
                    ins=[vag_in[:].opt()], outs=[vag_out[:].opt()])

                # --- Q projection + RoPE (overlaps K AllGather)
                t_q = wk_.tile([128, NJ * TPC], BF16, tag="q")
                wpass(wsl, di["Wq"], l, NJ // 2, t_xn,
                      lambda k: slice(k * TPC, (k + 1) * TPC),
                      lambda n, p: nc.scalar.activation(
                          t_q[:, n * TPC:(n + 1) * TPC], p, AF.Identity,
                          bias=bq_p[:, n:n + 1]))
                rope(wk_, t_q, t_cosQ, t_sinQ)

                # --- gather loads: clean [128, D]/[128, VW] copies per slot
                for j in range(NB):
                    r, hf = KB_RANK[j], KB_HALF[j]
                    nc.sync.dma_start(
                        t_K[:, j * D:(j + 1) * D],
                        kag_out[r * 128:(r + 1) * 128, hf * D:(hf + 1) * D])
                for j in range(NB):
                    r, hf = KB_RANK[j], KB_HALF[j]
                    eng = nc.sync if j % 2 == 0 else nc.scalar
                    eng.dma_start(
                        t_V[:, j * VW:(j + 1) * VW],
                        vag_out[r * 128:(r + 1) * 128, hf * VW:(hf + 1) * VW])

                # --- attention phase 1: QK + exp + mask for all heads
                t_As = []
                for h in range(H):
                    jq, po = h // 2, 64 * (h % 2)
                    t_A = ap_.tile([128, NB * TPC], BF16, tag="A",
                                   name=f"A{l}_{h}")
                    t_As.append(t_A)
                    for c in range(2):
                        ps_qk = ppqk.tile([128, 4 * TPC], F32, tag="qk",
                                          name=f"qk{l}_{h}_{c}")
                        for k4 in range(4):
                            kb = c * 4 + k4
                            nc.tensor.matmul(
                                ps_qk[:, k4 * TPC:(k4 + 1) * TPC],
                                t_K[po:po + 64,
                                    kb * D + jq * TB:kb * D + (jq + 1) * TB],
                                t_q[po:po + 64, jq * TPC:(jq + 1) * TPC])
                        nc.scalar.activation(
                            t_A[:, c * 4 * TPC:(c + 1) * 4 * TPC],
                            ps_qk[:], AF.Exp, scale=0.125)
                    eng = nc.vector if h % 2 == 0 else nc.gpsimd
                    eng.tensor_tensor(t_A[:], t_A[:], t_mask[:], OP.mult)

                # --- attention phase 2: AV + normalize, fused with Wo
                t_attT = wk_.tile([128, NJ * TPC], BF16, tag="attT")
                wo_pss = [psum2() for _ in range(3)]
                wo_ap = lambda n: wo_pss[n // 2][:, (n % 2) * TPC:(n % 2 + 1) * TPC]
                for jp in range(NJ):
                    ps_av = ppav.tile([128, 512], F32, tag="av", name=f"av{l}_{jp}")
                    for hh in range(2):
                        h = 2 * jp + hh
                        t_A = t_As[h]
                        for kb in range(NB):
                            nc.tensor.matmul(
                                ps_av[0:66, hh * TPC:(hh + 1) * TPC],
                                t_V[:, kb * VW + h * 66:kb * VW + h * 66 + 66],
                                t_A[:, kb * TPC:(kb + 1) * TPC],
                                start=(kb == 0 and hh == 0),
                                stop=(kb == NB - 1), skip_group_check=True)
                    for hh in range(2):
                        h = 2 * jp + hh
                        jq, po = h // 2, 64 * (h % 2)
                        t_den = small.tile([1, TPC], F32, tag="den")
                        nc.vector.tensor_copy(
                            t_den[:], ps_av[64:65, hh * TPC:(hh + 1) * TPC])
                        t_rl = small.tile([1, TPC], F32, tag="rl")
                        nc.vector.reciprocal_approx_fast(t_rl[:], t_den[:])
                        t_rb2 = small.tile([64, TPC], F32, tag="rb2")
                        nc.gpsimd.partition_broadcast(t_rb2[:], t_rl[:])
                        nc.vector.tensor_tensor(
                            t_attT[po:po + 64, jq * TPC:(jq + 1) * TPC],
                            ps_av[0:64, hh * TPC:(hh + 1) * TPC],
                            t_rb2[:], OP.mult)
                    if jp % 2 == 1:
                        kp = jp // 2
                        wo_k = wsl.tile([128, 2 * NJ * 128], BF16, tag="wsl")
                        nc.sync.dma_start(
                            wo_k[:], di["Wo"].ap()[l, kp * 128:(kp + 1) * 128, :])
                        for half in range(2):
                            k = 2 * kp + half
                            for n in range(NJ):
                                nc.tensor.matmul(
                                    wo_ap(n),
                                    wo_k[:, half * D + n * 128:
                                         half * D + (n + 1) * 128],
                                    t_attT[:, k * TPC:(k + 1) * TPC],
                                    start=(k == 0 and n % 2 == 0),
                                    stop=(k == NJ - 1), skip_group_check=True)
                # Wo residual straight into x (no staging tile)
                for n in range(NJ):
                    sl = slice(n * TPC, (n + 1) * TPC)
                    nc.vector.tensor_tensor(t_x[:, sl], t_x[:, sl], wo_ap(n),
                                            OP.add)
                    nc.vector.tensor_scalar_add(t_x[:, sl], t_x[:, sl],
                                                bo_p[:, n:n + 1])

                # --- LN2 + MLP
                t_xn2 = wk_.tile([128, NJ * TPC], BF16, tag="xn2")
                layernorm(wk_, t_x, t_xn2, l2w_p, l2b_p)

                t_h1 = h1p.tile([128, NJ1 * TPC], BF16, tag="h1")
                for g in range(4):
                    psg = [psum2() for _ in range(3)]
                    gap = lambda n: psg[n // 2][:, (n % 2) * TPC:(n % 2 + 1) * TPC]
                    for kp in range(NJ // 2):
                        w1k = wsl.tile([128, 2 * NJ * 128], BF16, tag="wsl")
                        nc.sync.dma_start(
                            w1k[:], di["W1"].ap()[l, kp * 128:(kp + 1) * 128, g])
                        for half in range(2):
                            k = 2 * kp + half
                            for n in range(NJ):
                                nc.tensor.matmul(
                                    gap(n), w1k[:, half * D + n * 128:
                                                half * D + (n + 1) * 128],
                                    t_xn2[:, k * TPC:(k + 1) * TPC],
                                    start=(k == 0 and n % 2 == 0),
                                    stop=(k == NJ - 1), skip_group_check=True)
                    for n in range(NJ):
                        gn = g * NJ + n
                        nc.scalar.activation(
                            t_h1[:, gn * TPC:(gn + 1) * TPC], gap(n), AF.Gelu,
                            bias=b1_p[:, gn:gn + 1])

                wpass(wsl, di["W2"], l, NJ1 // 2, t_h1,
                      lambda k: slice(k * TPC, (k + 1) * TPC),
                      lambda n, p: (
                          nc.vector.tensor_tensor(
                              t_x[:, n * TPC:(n + 1) * TPC],
                              t_x[:, n * TPC:(n + 1) * TPC], p, OP.add),
                          nc.vector.tensor_scalar_add(
                              t_x[:, n * TPC:(n + 1) * TPC],
                              t_x[:, n * TPC:(n + 1) * TPC],
                              b2_p[:, n:n + 1])))

        # ================= phase B: final LN + classifier =================
        with ExitStack() as bctx:
            bw = bctx.enter_context(tc.tile_pool(name="bw", bufs=1))
            hallp = bctx.enter_context(tc.tile_pool(name="hall", bufs=1))
            embp = bctx.enter_context(tc.tile_pool(name="embp", bufs=12))
            ppc = bctx.enter_context(tc.tile_pool(name="ppc", bufs=2, space="PSUM"))

            t_hT = bw.tile([128, NJ * TPC], BF16, tag="hT")
            layernorm(bw, t_x, t_hT, t_lnw, t_lnb)
            hag_in = dram.tile([128, NJ * TPC], BF16, tag="hag_in")
            nc.gpsimd.dma_start(hag_in[:], t_hT[:])
            hag_out = dram.tile([8 * 128, NJ * TPC], BF16, tag="hag_out",
                                addr_space="Shared")
            nc.gpsimd.collective_compute(
                "AllGather", OP.bypass,
                replica_groups=[[0, 1, 2, 3, 4, 5, 6, 7]],
                ins=[hag_in[:].opt()], outs=[hag_out[:].opt()])

            t_hall = hallp.tile([128, 8 * NJ * TPC], BF16, tag="hall")
            for r in range(8):
                eng = (nc.sync, nc.scalar, nc.gpsimd)[r % 3]
                eng.dma_start(t_hall[:, r * NJ * TPC:(r + 1) * NJ * TPC],
                              hag_out[r * 128:(r + 1) * 128, :])

            for vc in range(NVC):
                ets = []
                for k in range(NJ):
                    et = embp.tile([128, VCHUNK], BF16, tag="emb", name=f"emb{vc}_{k}")
                    nc.sync.dma_start(
                        et[:], di["embT"].ap()[k * 128:(k + 1) * 128,
                                               vc * VCHUNK:(vc + 1) * VCHUNK])
                    ets.append(et)
                for mt in range(NMT):
                    beta, j = divmod(mt, NB)
                    r, hf = beta * 4 + KB_RANK[j], KB_HALF[j]
                    pcnt[0] += 1
                    pc = ppc.tile([128, VCHUNK], F32, tag="ppc",
                                  name=f"pc{pcnt[0]}")
                    for k in range(NJ):
                        nc.tensor.matmul(
                            pc[:],
                            t_hall[:, (r * NJ + k) * TPC + hf * TB:
                                   (r * NJ + k) * TPC + (hf + 1) * TB],
                            ets[k][:], start=(k == 0), stop=(k == NJ - 1))
                    so = embp.tile([128, VCHUNK], F32, tag="clso",
                                   name=f"clso{vc}_{mt}")
                    if mt % 2 == 0:
                        nc.scalar.activation(so[:], pc[:], AF.Copy)
                    else:
                        nc.vector.tensor_copy(so[:], pc[:])
                    nc.sync.dma_start(
                        out_logits.ap()[mt * 128:(mt + 1) * 128,
                                        vc * VCHUNK:(vc + 1) * VCHUNK], so[:])

    nc.compile()
    return nc


_NC = None


def _get_nc():
    global _NC
    if _NC is None:
        _NC = _build()
    return _NC


def _pack_fm(M):
    """[768, t] feature-major -> [128, 6*t] tile layout (row d=128*j+p)."""
    t = M.shape[1]
    return np.ascontiguousarray(
        M.reshape(NJ, 128, t).transpose(1, 0, 2).reshape(128, NJ * t),
        dtype=np.float32)


def _pack_pp(v):
    """per-feature vector [D'] -> per-partition [128, D'/128]."""
    return np.ascontiguousarray(v.reshape(-1, 128).T, dtype=np.float32)


def _pair_pack(W):
    """[L, Dk, N] -> [L, Dk//2, 2N]: k-blocks (2kp, 2kp+1) side by side."""
    Lw, Dk, N = W.shape
    return np.ascontiguousarray(
        W.reshape(Lw, Dk // 256, 2, 128, N).transpose(0, 1, 3, 2, 4)
        .reshape(Lw, Dk // 2, 2 * N))


def _prep_in_maps(inputs):
    import ml_dtypes
    bf16 = ml_dtypes.bfloat16
    f32 = lambda a: np.ascontiguousarray(a, dtype=np.float32)
    emb = f32(inputs["emb"])
    tok = np.asarray(inputs["input_token"]).astype(np.int64)
    x0 = emb[tok]                                    # [B, T, D]

    params = np.zeros((L, 128, NPAR), np.float32)
    for li in range(L):
        cols = []
        for src in ("bq", "bk", "bo", "b2", "gamma", "beta", "ln2_w", "ln2_b"):
            cols.append(_pack_pp(f32(inputs[src][li])))
        cols.append(_pack_pp(f32(inputs["b1"][li])))
        cols.append(np.tile(f32(inputs["bv"][li])[None, :], (128, 1)))
        params[li] = np.concatenate(cols, axis=1)

    w1 = f32(inputs["W1"])  # [L, D, 4D] -> [L, D//2, 4, 2*768]
    w1p = (w1.reshape(L, D // 256, 2, 128, 4, D)
           .transpose(0, 1, 3, 4, 2, 5).reshape(L, D // 2, 4, 2 * D))

    shared = {
        "Wq": _pair_pack(f32(inputs["Wq"])).astype(bf16),
        "Wk": _pair_pack(f32(inputs["Wk"])).astype(bf16),
        "Wv": _pair_pack(f32(inputs["Wv"])).astype(bf16),
        "Wo": _pair_pack(f32(inputs["Wo"])).astype(bf16),
        "W1": np.ascontiguousarray(w1p).astype(bf16),
        "W2": _pair_pack(f32(inputs["W2"])).astype(bf16),
        "params": params,
        "onecol": np.full((128, 1), 1.0 / D, np.float32),
        "ones_v": np.ones((128, 2 * H, 2), bf16),
        "lnw_p": _pack_pp(f32(inputs["ln_w"])),
        "lnb_p": _pack_pp(f32(inputs["ln_b"])),
    }

    inv = 1.0 / (10000.0 ** (np.arange(0, DK, 2, dtype=np.float32) / DK))
    embT_full = emb.T                                # [D, V]
    vpad = np.zeros((D, 8 * VC), np.float32)
    vpad[:, :V] = embT_full

    # diag causal mask (key-major): M[kt, qt] = 1 if kt <= qt
    diag = np.tril(np.ones((TB, TB), np.float32)).T

    in_maps = []
    for c in range(8):
        beta, i = divmod(c, 4)
        qb = (i, 7 - i)
        pos = np.concatenate([np.arange(qb[0] * TB, (qb[0] + 1) * TB),
                              np.arange(qb[1] * TB, (qb[1] + 1) * TB)])
        xc = x0[beta, pos]                           # [256, D]
        m = dict(shared)
        m["x0T"] = _pack_fm(xc.T)

        fr = pos[:, None].astype(np.float32) * inv[None, :]      # [256, 32]
        ang = np.concatenate([fr, fr], 1)                        # [256, 64]
        cosT = np.cos(ang).T                                     # [64, 256]
        sinT = np.sin(ang).T
        sinSg = sinT.copy()
        sinSg[:32] = -sinT[:32]
        # Q layout: (j, hf, t) — identical 256-col block per j
        m["cosQ"] = np.ascontiguousarray(np.tile(cosT, (2, NJ))).astype(bf16)
        m["sinQ"] = np.ascontiguousarray(np.tile(sinSg, (2, NJ))).astype(bf16)
        # K layout: (hf, j, t) — per half: 128-col block tiled over j
        cos2 = np.tile(cosT, (2, 1))                             # [128, 256]
        sin2 = np.tile(sinSg, (2, 1))
        m["cosK"] = np.ascontiguousarray(np.concatenate(
            [np.tile(cos2[:, hf * TB:(hf + 1) * TB], (1, NJ)) for hf in (0, 1)],
            axis=1)).astype(bf16)
        m["sinK"] = np.ascontiguousarray(np.concatenate(
            [np.tile(sin2[:, hf * TB:(hf + 1) * TB], (1, NJ)) for hf in (0, 1)],
            axis=1)).astype(bf16)

        masks = np.zeros((NB, 128, TPC), np.float32)
        for kb in range(NB):
            for qi in range(2):
                blk = qb[qi]
                if kb < blk:
                    masks[kb, :, qi * TB:(qi + 1) * TB] = 1.0
                elif kb == blk:
                    masks[kb, :, qi * TB:(qi + 1) * TB] = diag
        m["masks"] = masks.astype(bf16)

        esl = np.zeros((D, VCP), np.float32)
        esl[:, :VC] = vpad[:, c * VC:(c + 1) * VC]
        m["embT"] = esl.astype(bf16)
        in_maps.append(m)

    return in_maps


def _assemble(res):
    out = np.empty((B, T, 8 * VC), np.float32)
    for c in range(8):
        lr = res.results[c]["logits"].reshape(B, T, VCP)
        out[:, :, c * VC:(c + 1) * VC] = lr[:, :, :VC]
    return np.ascontiguousarray(out[:, :, :V])


def kernel(**inputs):
    nc = _get_nc()
    in_maps = _prep_in_maps(inputs)
    res = bass_utils.run_bass_kernel_spmd(nc, in_maps, core_ids=list(range(8)))
    return _assemble(res)


def run_traced(inputs, tmpdir):
    nc = _get_nc()
    in_maps = _prep_in_maps(inputs)
    return bass_utils.run_bass_kernel_spmd(
        nc, in_maps, core_ids=list(range(8)), trace=True, tmpdir=tmpdir)


# revision 27
# speedup vs baseline: 1.3174x; 1.1264x over previous
"""GPT forward (L=6, B=2, T=1024, D=768, H=12, V=50257) on 8 TRN2 NeuronCores.

Sharding: tokens sharded 8-way (each core owns two causally-complementary
128-token blocks of one batch), weights replicated and streamed bf16 with
pair-packed k-tiles (3KB DMA runs), per-layer split K then V AllGathers
(bf16) within each 4-core batch group; V's AllGather overlaps the QK/exp
phase, which only needs K.  Classifier is vocab-sharded 8-way after a final
bf16 hidden-state AllGather.  Activations are feature-major [D, t]; all
matmuls are bf16 x bf16 into fp32 PSUM.  AV is feature-major (out[dk, q]) so
no PE transposes are needed; softmax denominators ride as an extra ones
column in the V payload.  The program is core-uniform: per-core differences
(token positions, causal masks, vocab slice) enter as input data.
"""
import os
import numpy as np
from contextlib import ExitStack

import concourse.bass as bass
import concourse.tile as tile
import concourse.mybir as mybir
from concourse import bacc, bass_utils

F32 = mybir.dt.float32
F32R = mybir.dt.float32r
BF16 = mybir.dt.bfloat16
AF = mybir.ActivationFunctionType
OP = mybir.AluOpType

L, B, T, D, H, DK, V = 6, 2, 1024, 768, 12, 64, 50257
NB, TB, TPC = 8, 128, 256
NJ = D // 128                        # 6
NJ1 = 4 * D // 128                   # 24
VCHUNK = 512
NVC = 13
VCP = NVC * VCHUNK                   # 6656
VC = 6283                            # 8*6283 = 50264 >= V
EPS = 1e-5
NMT = 16
VW = H * 66                          # 792: one half's V payload incl ones
NPAR = 8 * NJ + NJ1 + D              # packed per-layer params: 48+24+768
NLAYER = int(os.environ.get("KLAYERS", str(L)))

KB_RANK = [j if j < 4 else 7 - j for j in range(NB)]
KB_HALF = [0 if j < 4 else 1 for j in range(NB)]


def _build():
    nc = bacc.Bacc("TRN2", target_bir_lowering=False, debug=False)

    di = {}
    def din(name, shape, dt=F32R):
        di[name] = nc.dram_tensor(name, shape, dt, kind="ExternalInput")
        return di[name]

    din("x0T", [128, NJ * TPC])
    din("cosQ", [128, NJ * TPC], BF16)
    din("sinQ", [128, NJ * TPC], BF16)
    din("cosK", [128, NJ * TPC], BF16)
    din("sinK", [128, NJ * TPC], BF16)
    din("masks", [NB, 128, TPC], BF16)
    din("onecol", [128, 1])
    din("ones_v", [128, 2 * H, 2], BF16)
    din("embT", [D, VCP], BF16)
    for nm in ("Wq", "Wk", "Wv", "Wo"):
        din(nm, [L, D // 2, 2 * D], BF16)      # pair-packed k-tiles
    din("W1", [L, D // 2, 4, 2 * D], BF16)     # (kp, p, group, half*n)
    din("W2", [L, 2 * D, 2 * D], BF16)
    din("params", [L, 128, NPAR], F32)
    din("lnw_p", [128, NJ], F32)
    din("lnb_p", [128, NJ], F32)

    out_logits = nc.dram_tensor("logits", [NMT * 128, VCP], F32, kind="ExternalOutput")

    with tile.TileContext(nc) as tc, ExitStack() as octx:
        const = octx.enter_context(tc.tile_pool(name="const", bufs=1))
        xpool = octx.enter_context(tc.tile_pool(name="x", bufs=1))
        small = octx.enter_context(tc.tile_pool(name="small", bufs=2))
        bias = octx.enter_context(tc.tile_pool(name="bias", bufs=2))
        pp = octx.enter_context(tc.tile_pool(name="pp", bufs=3, space="PSUM"))
        dram = octx.enter_context(tc.tile_pool(name="dram", bufs=2, space="DRAM"))

        t_ones = const.tile([128, 1], F32R, tag="ones")
        nc.sync.dma_start(t_ones[:], di["onecol"].ap())
        t_lnw = const.tile([128, NJ], F32, tag="lnw")
        nc.sync.dma_start(t_lnw[:], di["lnw_p"].ap())
        t_lnb = const.tile([128, NJ], F32, tag="lnb")
        nc.sync.dma_start(t_lnb[:], di["lnb_p"].ap())
        t_eps = const.tile([1, 1], F32, tag="eps")
        nc.gpsimd.memset(t_eps[:], EPS)

        t_x = xpool.tile([128, NJ * TPC], F32R, tag="x")
        nc.sync.dma_start(t_x[:], di["x0T"].ap())
        t_da = const.tile([128, TPC], F32R, tag="dwa")
        nc.sync.dma_start(t_da[:], di["x0T"].ap()[:, 0:TPC])
        t_db = const.tile([128, TPC], F32R, tag="dwb")
        nc.sync.dma_start(t_db[:], di["x0T"].ap()[:, 0:TPC])

        pcnt = [0]

        def keep_warm(pool, nlinks, nm):
            # PE/DVE ping-pong: one tiny matmul every ~1us keeps the HAM
            # clock-gate at 8/8 across a wait that has no real PE work.
            pcnt[0] += 1
            t_dw = pool.tile([1, 512], F32, tag="pp", name=f"dw{nm}{pcnt[0]}")
            for i in range(nlinks):
                srct, dstt = (t_da, t_db) if i % 2 == 0 else (t_db, t_da)
                sl = t_dw[:, (i % 2) * TPC:(i % 2 + 1) * TPC]
                nc.tensor.matmul(sl, t_ones[:], srct[:],
                                 skip_group_check=True)
                nc.vector.tensor_copy(dstt[0:1, :], sl)

        def psum2():
            pcnt[0] += 1
            return pp.tile([128, 512], F32, tag="pp", name=f"ps{pcnt[0]}")

        def layernorm(wpool, src, dst, gt, bt):
            """feature-major LN: dst = (src - mean)/std * g + b, per token."""
            pcnt[0] += 1
            t_red = pp.tile([1, 512], F32, tag="pp", name=f"red{pcnt[0]}")
            p_s, p_q = t_red[:, 0:TPC], t_red[:, TPC:2 * TPC]
            for j in range(NJ):
                nc.tensor.matmul(p_s, t_ones[:], src[:, j * TPC:(j + 1) * TPC],
                                 start=(j == 0), stop=(j == NJ - 1))
            t_sq = wpool.tile([128, NJ * TPC], F32R, tag="scratch6")
            nc.vector.tensor_tensor(t_sq[:], src[:], src[:], OP.mult)
            for j in range(NJ):
                nc.tensor.matmul(p_q, t_ones[:], t_sq[:, j * TPC:(j + 1) * TPC],
                                 start=(j == 0), stop=(j == NJ - 1),
                                 skip_group_check=True)
            # ones vector holds 1/D, so p_s = mean and p_q = E[x^2] directly
            t_mean = small.tile([1, TPC], F32, tag="mean")
            nc.vector.tensor_copy(t_mean[:], p_s)
            t_msq = small.tile([1, TPC], F32, tag="msq")
            nc.vector.tensor_tensor(t_msq[:], t_mean[:], t_mean[:], OP.mult)
            t_var = small.tile([1, TPC], F32, tag="var")
            nc.vector.tensor_tensor(t_var[:], p_q, t_msq[:], OP.subtract)
            t_std = small.tile([1, TPC], F32, tag="std")
            nc.scalar.activation(t_std[:], t_var[:], AF.Sqrt, bias=t_eps[:])
            t_rstd = small.tile([1, TPC], F32, tag="rstd")
            nc.vector.reciprocal_approx_fast(t_rstd[:], t_std[:])
            t_mb = small.tile([128, TPC], F32, tag="mb")
            nc.gpsimd.partition_broadcast(t_mb[:], t_mean[:])
            t_rb = small.tile([128, TPC], F32, tag="rb")
            nc.gpsimd.partition_broadcast(t_rb[:], t_rstd[:])
            for j in range(NJ):
                sl = slice(j * TPC, (j + 1) * TPC)
                eng = nc.vector if j % 2 == 0 else nc.gpsimd
                eng.tensor_tensor(t_sq[:, sl], src[:, sl], t_mb[:], OP.subtract)
                eng.tensor_tensor(t_sq[:, sl], t_sq[:, sl], t_rb[:], OP.mult)
                eng.tensor_scalar(dst[:, sl], t_sq[:, sl], gt[:, j:j + 1],
                                  bt[:, j:j + 1], OP.mult, OP.add)

        def rope(wpool, t_r, t_cos, t_sin):
            """in-place RoPE on feature-major bf16 [128, NJ*TPC] tile."""
            t_sw = wpool.tile([128, NJ * TPC], BF16, tag="ropesw")
            W = NJ * TPC
            nc.vector.tensor_copy(t_sw[0:32, 0:W], t_r[32:64, 0:W])
            nc.vector.tensor_copy(t_sw[32:64, 0:W], t_r[0:32, 0:W])
            nc.vector.tensor_copy(t_sw[64:96, 0:W], t_r[96:128, 0:W])
            nc.vector.tensor_copy(t_sw[96:128, 0:W], t_r[64:96, 0:W])
            nc.vector.tensor_tensor(t_sw[:], t_sw[:], t_sin[:], OP.mult)
            nc.vector.tensor_tensor(t_r[:], t_r[:], t_cos[:], OP.mult)
            nc.vector.tensor_tensor(t_r[:], t_r[:], t_sw[:], OP.add)

        def wpass(wsl_pool, wdram, l, nkp, rhs, rhs_k_slice, out_fn):
            """out[n] = sum_k W[l,k].T @ rhs_k with pair-packed weight tiles.
            wdram tile kp holds k-blocks (2kp, 2kp+1) side by side [128, 2N].
            out_fn(n, ps_ap) evicts a [128, TPC] psum AP for feature-tile n."""
            pss = [psum2() for _ in range(3)]
            pap = lambda n: pss[n // 2][:, (n % 2) * TPC:(n % 2 + 1) * TPC]
            for kp in range(nkp):
                wk = wsl_pool.tile([128, 2 * NJ * 128], BF16, tag="wsl")
                nc.sync.dma_start(wk[:], wdram.ap()[l, kp * 128:(kp + 1) * 128, :])
                for half in range(2):
                    k = 2 * kp + half
                    for n in range(NJ):
                        nc.tensor.matmul(
                            pap(n),
                            wk[:, half * D + n * 128: half * D + (n + 1) * 128],
                            rhs[:, rhs_k_slice(k)],
                            start=(k == 0 and n % 2 == 0),
                            stop=(k == 2 * nkp - 1),
                            skip_group_check=True)
            for n in range(NJ):
                out_fn(n, pap(n))

        # ================= phase A: transformer layers =================
        with ExitStack() as actx:
            aconst = actx.enter_context(tc.tile_pool(name="aconst", bufs=1))
            kvp = actx.enter_context(tc.tile_pool(name="kvp", bufs=1))
            wk_ = actx.enter_context(tc.tile_pool(name="work", bufs=1))
            ap_ = actx.enter_context(tc.tile_pool(name="Ap", bufs=12))
            wsl = actx.enter_context(tc.tile_pool(name="wsl", bufs=6))
            h1p = actx.enter_context(tc.tile_pool(name="h1p", bufs=1))
            ppqk = actx.enter_context(
                tc.tile_pool(name="ppqk", bufs=2, space="PSUM"))
            ppav = actx.enter_context(
                tc.tile_pool(name="ppav", bufs=1, space="PSUM"))

            t_cosQ = aconst.tile([128, NJ * TPC], BF16, tag="cosQ")
            nc.sync.dma_start(t_cosQ[:], di["cosQ"].ap())
            t_sinQ = aconst.tile([128, NJ * TPC], BF16, tag="sinQ")
            nc.sync.dma_start(t_sinQ[:], di["sinQ"].ap())
            t_cosK = aconst.tile([128, NJ * TPC], BF16, tag="cosK")
            nc.sync.dma_start(t_cosK[:], di["cosK"].ap())
            t_sinK = aconst.tile([128, NJ * TPC], BF16, tag="sinK")
            nc.sync.dma_start(t_sinK[:], di["sinK"].ap())
            t_mask = aconst.tile([128, NB * TPC], BF16, tag="mask")
            for kb in range(NB):
                nc.sync.dma_start(t_mask[:, kb * TPC:(kb + 1) * TPC],
                                  di["masks"].ap()[kb])

            t_K = kvp.tile([128, NB * NJ * TB], BF16, tag="K")    # (slot, j, t)
            t_V = kvp.tile([128, NB * VW], BF16, tag="V")         # (slot, h, dk|1|1)
            # producer-side V payload, hf-major with inline ones columns
            t_vc2 = kvp.tile([128, 2 * VW], BF16, tag="vc2")
            nc.sync.dma_start(
                t_vc2[:].rearrange("p (g e) -> p g e", g=2 * H)[:, :, 64:66],
                di["ones_v"].ap())

            for l in range(NLAYER):
                # --- packed per-layer params, one DMA
                t_par = bias.tile([128, NPAR], F32, tag="par")
                nc.sync.dma_start(t_par[:], di["params"].ap()[l])
                bq_p = t_par[:, 0 * NJ:1 * NJ]
                bk_p = t_par[:, 1 * NJ:2 * NJ]
                bo_p = t_par[:, 2 * NJ:3 * NJ]
                b2_p = t_par[:, 3 * NJ:4 * NJ]
                g_p = t_par[:, 4 * NJ:5 * NJ]
                be_p = t_par[:, 5 * NJ:6 * NJ]
                l2w_p = t_par[:, 6 * NJ:7 * NJ]
                l2b_p = t_par[:, 7 * NJ:8 * NJ]
                b1_p = t_par[:, 8 * NJ:8 * NJ + NJ1]
                bv_bc = t_par[:, 8 * NJ + NJ1:]

                # --- LN1
                t_xn = wk_.tile([128, NJ * TPC], BF16, tag="xn")
                layernorm(wk_, t_x, t_xn, g_p, be_p)

                # --- K projection (evict hf-major) + RoPE, then K AllGather
                t_k = wk_.tile([128, NJ * TPC], BF16, tag="k")
                kview = t_k[:].rearrange("p (hf j t) -> p hf j t", hf=2, j=NJ)
                wpass(wsl, di["Wk"], l, NJ // 2, t_xn,
                      lambda k: slice(k * TPC, (k + 1) * TPC),
                      lambda n, p: nc.scalar.activation(
                          kview[:, :, n, :], p, AF.Identity,
                          bias=bk_p[:, n:n + 1]))
                rope(wk_, t_k, t_cosK, t_sinK)
                kag_in = dram.tile([128, NJ * TPC], BF16, tag="kag_in")
                nc.gpsimd.dma_start(kag_in[:], t_k[:])
                kag_out = dram.tile([4 * 128, NJ * TPC], BF16, tag="kag_out")
                nc.gpsimd.collective_compute(
                    "AllGather", OP.bypass,
                    replica_groups=[[0, 1, 2, 3], [4, 5, 6, 7]],
                    ins=[kag_in[:].opt()], outs=[kag_out[:].opt()])

                # --- V projection (token-major) + bias -> t_vc2, V AllGather
                psv = [psum2() for _ in range(3)]
                vap = lambda i: psv[i // 2][:, (i % 2) * TPC:(i % 2 + 1) * TPC]
                for kp in range(NJ // 2):
                    wvk = wsl.tile([128, 2 * NJ * 128], BF16, tag="wsl")
                    nc.sync.dma_start(
                        wvk[:], di["Wv"].ap()[l, kp * 128:(kp + 1) * 128, :])
                    for half in range(2):
                        k = 2 * kp + half
                        for tt in range(2):
                            lhs = t_xn[:, k * TPC + tt * TB: k * TPC + (tt + 1) * TB]
                            for s3 in range(3):
                                i6 = tt * 3 + s3
                                nc.tensor.matmul(
                                    vap(i6), lhs,
                                    wvk[:, half * D + s3 * 256:
                                        half * D + (s3 + 1) * 256],
                                    start=(k == 0 and i6 % 2 == 0),
                                    stop=(k == NJ - 1), skip_group_check=True)
                vc2v = t_vc2[:].rearrange("p (hf g e) -> p hf g e", hf=2, g=H)
                for tt in range(2):
                    for s3 in range(3):
                        nc.vector.tensor_tensor(
                            vc2v[:, tt, s3 * 4:(s3 + 1) * 4, 0:64],
                            vap(tt * 3 + s3).rearrange("p (g e) -> p g e", g=4),
                            bv_bc[:, s3 * 256:(s3 + 1) * 256]
                            .rearrange("p (g e) -> p g e", g=4), OP.add)
                vag_in = dram.tile([128, 2 * VW], BF16, tag="vag_in")
                nc.gpsimd.dma_start(vag_in[:], t_vc2[:])
                vag_out = dram.tile([4 * 128, 2 * VW], BF16, tag="vag_out")
                nc.gpsimd.collective_compute(
                    "AllGather", OP.bypass,
                    replica_groups=[[0, 1, 2, 3], [4, 5, 6, 7]],
                    ins=[vag_in[:].opt()], outs=[vag_out[:].opt()])

                # --- Q projection + RoPE (overlaps K AllGather)
                t_q = wk_.tile([128, NJ * TPC], BF16, tag="q")
                wpass(wsl, di["Wq"], l, NJ // 2, t_xn,
                      lambda k: slice(k * TPC, (k + 1) * TPC),
                      lambda n, p: nc.scalar.activation(
                          t_q[:, n * TPC:(n + 1) * TPC], p, AF.Identity,
                          bias=bq_p[:, n:n + 1]))
                rope(wk_, t_q, t_cosQ, t_sinQ)

                # --- gather loads: clean [128, D]/[128, VW] copies per slot
                for j in range(NB):
                    r, hf = KB_RANK[j], KB_HALF[j]
                    nc.sync.dma_start(
                        t_K[:, j * D:(j + 1) * D],
                        kag_out[r * 128:(r + 1) * 128, hf * D:(hf + 1) * D])
                for j in range(NB):
                    r, hf = KB_RANK[j], KB_HALF[j]
                    eng = nc.sync if j % 2 == 0 else nc.scalar
                    eng.dma_start(
                        t_V[:, j * VW:(j + 1) * VW],
                        vag_out[r * 128:(r + 1) * 128, hf * VW:(hf + 1) * VW])

                keep_warm(pp, 14, f"ag{l}")

                # --- attention phase 1: QK + exp + mask for all heads
                t_As = []
                for h in range(H):
                    jq, po = h // 2, 64 * (h % 2)
                    t_A = ap_.tile([128, NB * TPC], BF16, tag="A",
                                   name=f"A{l}_{h}")
                    t_As.append(t_A)
                    for c in range(2):
                        ps_qk = ppqk.tile([128, 4 * TPC], F32, tag="qk",
                                          name=f"qk{l}_{h}_{c}")
                        for k4 in range(4):
                            kb = c * 4 + k4
                            nc.tensor.matmul(
                                ps_qk[:, k4 * TPC:(k4 + 1) * TPC],
                                t_K[po:po + 64,
                                    kb * D + jq * TB:kb * D + (jq + 1) * TB],
                                t_q[po:po + 64, jq * TPC:(jq + 1) * TPC])
                        nc.scalar.activation(
                            t_A[:, c * 4 * TPC:(c + 1) * 4 * TPC],
                            ps_qk[:], AF.Exp, scale=0.125)
                    nc.vector.tensor_tensor(t_A[:], t_A[:], t_mask[:], OP.mult)

                # --- attention phase 2: AV + normalize, fused with Wo
                t_attT = wk_.tile([128, NJ * TPC], BF16, tag="attT")
                wo_pss = [psum2() for _ in range(3)]
                wo_ap = lambda n: wo_pss[n // 2][:, (n % 2) * TPC:(n % 2 + 1) * TPC]
                for jp in range(NJ):
                    ps_av = ppav.tile([128, 512], F32, tag="av", name=f"av{l}_{jp}")
                    for hh in range(2):
                        h = 2 * jp + hh
                        t_A = t_As[h]
                        for kb in range(NB):
                            nc.tensor.matmul(
                                ps_av[0:66, hh * TPC:(hh + 1) * TPC],
                                t_V[:, kb * VW + h * 66:kb * VW + h * 66 + 66],
                                t_A[:, kb * TPC:(kb + 1) * TPC],
                                start=(kb == 0 and hh == 0),
                                stop=(kb == NB - 1), skip_group_check=True)
                    for hh in range(2):
                        h = 2 * jp + hh
                        jq, po = h // 2, 64 * (h % 2)
                        t_den = small.tile([1, TPC], F32, tag="den")
                        nc.vector.tensor_copy(
                            t_den[:], ps_av[64:65, hh * TPC:(hh + 1) * TPC])
                        t_rl = small.tile([1, TPC], F32, tag="rl")
                        nc.vector.reciprocal_approx_fast(t_rl[:], t_den[:])
                        t_rb2 = small.tile([64, TPC], F32, tag="rb2")
                        nc.gpsimd.partition_broadcast(t_rb2[:], t_rl[:])
                        nc.vector.tensor_tensor(
                            t_attT[po:po + 64, jq * TPC:(jq + 1) * TPC],
                            ps_av[0:64, hh * TPC:(hh + 1) * TPC],
                            t_rb2[:], OP.mult)
                    if jp % 2 == 1:
                        kp = jp // 2
                        wo_k = wsl.tile([128, 2 * NJ * 128], BF16, tag="wsl")
                        nc.sync.dma_start(
                            wo_k[:], di["Wo"].ap()[l, kp * 128:(kp + 1) * 128, :])
                        for half in range(2):
                            k = 2 * kp + half
                            for n in range(NJ):
                                nc.tensor.matmul(
                                    wo_ap(n),
                                    wo_k[:, half * D + n * 128:
                                         half * D + (n + 1) * 128],
                                    t_attT[:, k * TPC:(k + 1) * TPC],
                                    start=(k == 0 and n % 2 == 0),
                                    stop=(k == NJ - 1), skip_group_check=True)
                # Wo residual straight into x (no staging tile)
                for n in range(NJ):
                    sl = slice(n * TPC, (n + 1) * TPC)
                    nc.vector.tensor_tensor(t_x[:, sl], t_x[:, sl], wo_ap(n),
                                            OP.add)
                    nc.vector.tensor_scalar_add(t_x[:, sl], t_x[:, sl],
                                                bo_p[:, n:n + 1])

                # --- LN2 + MLP
                t_xn2 = wk_.tile([128, NJ * TPC], BF16, tag="xn2")
                layernorm(wk_, t_x, t_xn2, l2w_p, l2b_p)

                keep_warm(pp, 10, f"ln{l}")

                t_h1 = h1p.tile([128, NJ1 * TPC], BF16, tag="h1")
                for g in range(4):
                    psg = [psum2() for _ in range(3)]
                    gap = lambda n: psg[n // 2][:, (n % 2) * TPC:(n % 2 + 1) * TPC]
                    for kp in range(NJ // 2):
                        w1k = wsl.tile([128, 2 * NJ * 128], BF16, tag="wsl")
                        nc.sync.dma_start(
                            w1k[:], di["W1"].ap()[l, kp * 128:(kp + 1) * 128, g])
                        for half in range(2):
                            k = 2 * kp + half
                            for n in range(NJ):
                                nc.tensor.matmul(
                                    gap(n), w1k[:, half * D + n * 128:
                                                half * D + (n + 1) * 128],
                                    t_xn2[:, k * TPC:(k + 1) * TPC],
                                    start=(k == 0 and n % 2 == 0),
                                    stop=(k == NJ - 1), skip_group_check=True)
                    for n in range(NJ):
                        gn = g * NJ + n
                        nc.scalar.activation(
                            t_h1[:, gn * TPC:(gn + 1) * TPC], gap(n), AF.Gelu,
                            bias=b1_p[:, gn:gn + 1])

                wpass(wsl, di["W2"], l, NJ1 // 2, t_h1,
                      lambda k: slice(k * TPC, (k + 1) * TPC),
                      lambda n, p: (
                          nc.vector.tensor_tensor(
                              t_x[:, n * TPC:(n + 1) * TPC],
                              t_x[:, n * TPC:(n + 1) * TPC], p, OP.add),
                          nc.vector.tensor_scalar_add(
                              t_x[:, n * TPC:(n + 1) * TPC],
                              t_x[:, n * TPC:(n + 1) * TPC],
                              b2_p[:, n:n + 1])))

        # ================= phase B: final LN + classifier =================
        with ExitStack() as bctx:
            bw = bctx.enter_context(tc.tile_pool(name="bw", bufs=1))
            hallp = bctx.enter_context(tc.tile_pool(name="hall", bufs=1))
            embp = bctx.enter_context(tc.tile_pool(name="embp", bufs=12))
            ppc = bctx.enter_context(tc.tile_pool(name="ppc", bufs=2, space="PSUM"))

            t_hT = bw.tile([128, NJ * TPC], BF16, tag="hT")
            layernorm(bw, t_x, t_hT, t_lnw, t_lnb)
            hag_in = dram.tile([128, NJ * TPC], BF16, tag="hag_in")
            nc.gpsimd.dma_start(hag_in[:], t_hT[:])
            hag_out = dram.tile([8 * 128, NJ * TPC], BF16, tag="hag_out",
                                addr_space="Shared")
            nc.gpsimd.collective_compute(
                "AllGather", OP.bypass,
                replica_groups=[[0, 1, 2, 3, 4, 5, 6, 7]],
                ins=[hag_in[:].opt()], outs=[hag_out[:].opt()])

            t_hall = hallp.tile([128, 8 * NJ * TPC], BF16, tag="hall")
            for r in range(8):
                eng = (nc.sync, nc.scalar, nc.gpsimd)[r % 3]
                eng.dma_start(t_hall[:, r * NJ * TPC:(r + 1) * NJ * TPC],
                              hag_out[r * 128:(r + 1) * 128, :])

            for vc in range(NVC):
                ets = []
                for k in range(NJ):
                    et = embp.tile([128, VCHUNK], BF16, tag="emb", name=f"emb{vc}_{k}")
                    nc.sync.dma_start(
                        et[:], di["embT"].ap()[k * 128:(k + 1) * 128,
                                               vc * VCHUNK:(vc + 1) * VCHUNK])
                    ets.append(et)
                for mt in range(NMT):
                    beta, j = divmod(mt, NB)
                    r, hf = beta * 4 + KB_RANK[j], KB_HALF[j]
                    pcnt[0] += 1
                    pc = ppc.tile([128, VCHUNK], F32, tag="ppc",
                                  name=f"pc{pcnt[0]}")
                    for k in range(NJ):
                        nc.tensor.matmul(
                            pc[:],
                            t_hall[:, (r * NJ + k) * TPC + hf * TB:
                                   (r * NJ + k) * TPC + (hf + 1) * TB],
                            ets[k][:], start=(k == 0), stop=(k == NJ - 1))
                    so = embp.tile([128, VCHUNK], F32, tag="clso",
                                   name=f"clso{vc}_{mt}")
                    if mt % 2 == 0:
                        nc.scalar.activation(so[:], pc[:], AF.Copy)
                    else:
                        nc.vector.tensor_copy(so[:], pc[:])
                    nc.sync.dma_start(
                        out_logits.ap()[mt * 128:(mt + 1) * 128,
                                        vc * VCHUNK:(vc + 1) * VCHUNK], so[:])

    nc.compile()
    return nc


_NC = None


def _get_nc():
    global _NC
    if _NC is None:
        _NC = _build()
    return _NC


def _pack_fm(M):
    """[768, t] feature-major -> [128, 6*t] tile layout (row d=128*j+p)."""
    t = M.shape[1]
    return np.ascontiguousarray(
        M.reshape(NJ, 128, t).transpose(1, 0, 2).reshape(128, NJ * t),
        dtype=np.float32)


def _pack_pp(v):
    """per-feature vector [D'] -> per-partition [128, D'/128]."""
    return np.ascontiguousarray(v.reshape(-1, 128).T, dtype=np.float32)


def _pair_pack(W):
    """[L, Dk, N] -> [L, Dk//2, 2N]: k-blocks (2kp, 2kp+1) side by side."""
    Lw, Dk, N = W.shape
    return np.ascontiguousarray(
        W.reshape(Lw, Dk // 256, 2, 128, N).transpose(0, 1, 3, 2, 4)
        .reshape(Lw, Dk // 2, 2 * N))


def _prep_in_maps(inputs):
    import ml_dtypes
    bf16 = ml_dtypes.bfloat16
    f32 = lambda a: np.ascontiguousarray(a, dtype=np.float32)
    emb = f32(inputs["emb"])
    tok = np.asarray(inputs["input_token"]).astype(np.int64)
    x0 = emb[tok]                                    # [B, T, D]

    params = np.zeros((L, 128, NPAR), np.float32)
    for li in range(L):
        cols = []
        for src in ("bq", "bk", "bo", "b2", "gamma", "beta", "ln2_w", "ln2_b"):
            cols.append(_pack_pp(f32(inputs[src][li])))
        cols.append(_pack_pp(f32(inputs["b1"][li])))
        cols.append(np.tile(f32(inputs["bv"][li])[None, :], (128, 1)))
        params[li] = np.concatenate(cols, axis=1)

    w1 = f32(inputs["W1"])  # [L, D, 4D] -> [L, D//2, 4, 2*768]
    w1p = (w1.reshape(L, D // 256, 2, 128, 4, D)
           .transpose(0, 1, 3, 4, 2, 5).reshape(L, D // 2, 4, 2 * D))

    shared = {
        "Wq": _pair_pack(f32(inputs["Wq"])).astype(bf16),
        "Wk": _pair_pack(f32(inputs["Wk"])).astype(bf16),
        "Wv": _pair_pack(f32(inputs["Wv"])).astype(bf16),
        "Wo": _pair_pack(f32(inputs["Wo"])).astype(bf16),
        "W1": np.ascontiguousarray(w1p).astype(bf16),
        "W2": _pair_pack(f32(inputs["W2"])).astype(bf16),
        "params": params,
        "onecol": np.full((128, 1), 1.0 / D, np.float32),
        "ones_v": np.ones((128, 2 * H, 2), bf16),
        "lnw_p": _pack_pp(f32(inputs["ln_w"])),
        "lnb_p": _pack_pp(f32(inputs["ln_b"])),
    }

    inv = 1.0 / (10000.0 ** (np.arange(0, DK, 2, dtype=np.float32) / DK))
    embT_full = emb.T                                # [D, V]
    vpad = np.zeros((D, 8 * VC), np.float32)
    vpad[:, :V] = embT_full

    # diag causal mask (key-major): M[kt, qt] = 1 if kt <= qt
    diag = np.tril(np.ones((TB, TB), np.float32)).T

    in_maps = []
    for c in range(8):
        beta, i = divmod(c, 4)
        qb = (i, 7 - i)
        pos = np.concatenate([np.arange(qb[0] * TB, (qb[0] + 1) * TB),
                              np.arange(qb[1] * TB, (qb[1] + 1) * TB)])
        xc = x0[beta, pos]                           # [256, D]
        m = dict(shared)
        m["x0T"] = _pack_fm(xc.T)

        fr = pos[:, None].astype(np.float32) * inv[None, :]      # [256, 32]
        ang = np.concatenate([fr, fr], 1)                        # [256, 64]
        cosT = np.cos(ang).T                                     # [64, 256]
        sinT = np.sin(ang).T
        sinSg = sinT.copy()
        sinSg[:32] = -sinT[:32]
        # Q layout: (j, hf, t) — identical 256-col block per j
        m["cosQ"] = np.ascontiguousarray(np.tile(cosT, (2, NJ))).astype(bf16)
        m["sinQ"] = np.ascontiguousarray(np.tile(sinSg, (2, NJ))).astype(bf16)
        # K layout: (hf, j, t) — per half: 128-col block tiled over j
        cos2 = np.tile(cosT, (2, 1))                             # [128, 256]
        sin2 = np.tile(sinSg, (2, 1))
        m["cosK"] = np.ascontiguousarray(np.concatenate(
            [np.tile(cos2[:, hf * TB:(hf + 1) * TB], (1, NJ)) for hf in (0, 1)],
            axis=1)).astype(bf16)
        m["sinK"] = np.ascontiguousarray(np.concatenate(
            [np.tile(sin2[:, hf * TB:(hf + 1) * TB], (1, NJ)) for hf in (0, 1)],
            axis=1)).astype(bf16)

        masks = np.zeros((NB, 128, TPC), np.float32)
        for kb in range(NB):
            for qi in range(2):
                blk = qb[qi]
                if kb < blk:
                    masks[kb, :, qi * TB:(qi + 1) * TB] = 1.0
                elif kb == blk:
                    masks[kb, :, qi * TB:(qi + 1) * TB] = diag
        m["masks"] = masks.astype(bf16)

        esl = np.zeros((D, VCP), np.float32)
        esl[:, :VC] = vpad[:, c * VC:(c + 1) * VC]
        m["embT"] = esl.astype(bf16)
        in_maps.append(m)

    return in_maps


def _assemble(res):
    out = np.empty((B, T, 8 * VC), np.float32)
    for c in range(8):
        lr = res.results[c]["logits"].reshape(B, T, VCP)
        out[:, :, c * VC:(c + 1) * VC] = lr[:, :, :VC]
    return np.ascontiguousarray(out[:, :, :V])


def kernel(**inputs):
    nc = _get_nc()
    in_maps = _prep_in_maps(inputs)
    res = bass_utils.run_bass_kernel_spmd(nc, in_maps, core_ids=list(range(8)))
    return _assemble(res)


def run_traced(inputs, tmpdir):
    nc = _get_nc()
    in_maps = _prep_in_maps(inputs)
    return bass_utils.run_bass_kernel_spmd(
        nc, in_maps, core_ids=list(range(8)), trace=True, tmpdir=tmpdir)
